# revision 3
# baseline (speedup 1.0000x reference)
"""Single-head attention (B=8, S=2048, D_in=D_out=1024) on 8 Trainium2 NeuronCores.

Sharding: data-parallel over batch — core b computes batch element b end-to-end.
Weights (W_K/W_V/W_Q) are replicated to every core.

All matmul operands are bf16 (host-cast); PSUM accumulation is fp32. The PE
streams 1 cyc per output column for bf16 (same rate as fp32r), so matmul time
is unchanged vs the fp32r baseline, but bf16 unlocks the DMA XBAR transpose
(16x128 tiles @ 14ns, runs on the DMA engines) which removes ALL transposes
from the PE:
  - X^T [d, s] tiles are produced by DMA-transposing X straight out of DRAM.
  - P^T tiles are produced by SBUF->SBUF DMA transpose of the softmax rows.
The fp32r baseline spent ~70us of PE time on 640 identity-matmul transposes;
here the PE does only the 1792 "real" matmuls (~382us at 2.4 GHz).

Per-core program:
  Phase A (projections; contraction d on SBUF partitions via XBAR X^T):
    K^T = accum_d W_k[d]^T @ X_k^T[d]  -> 8 tiles [128 e, 2048 j]   (SBUF, bf16)
    Q^T = accum_d W_q[d]^T @ X_q^T[d]  -> 8 tiles [128 e, 2048 i]   (SBUF, bf16)
    V   = accum_d X_v^T[d]^T @ W_v[d]  -> 16 tiles [128 j, 1024 e]  (SBUF, bf16)
    (Q^T is SBUF-resident in bf16 — no DRAM scratch round-trip.)
  Phase B (attention, per 128-query tile, software-pipelined so the PE
  stream is [qk(0)][qk(1)][pv(0)][qk(2)][pv(1)]... and exp/transpose latency
  of tile it hides under qk(it+1)):
    S chunk [128 i, 512 j] = accum_e qt[e][:,it].T @ kt[e][:,chunk]  (PSUM)
    P chunk = exp(S/32) on ACT (bf16 out) with fused row-sum accumulation.
      No max subtraction: scores are O(+-17), exp stays inside fp32/bf16
      range, softmax is shift-invariant.
    P^T tiles [128 j, 128 i] via DMA XBAR (112ns each, off the PE)
    Z [128 i, 1024 e] = accum_j pt[j].T @ v[j]                       (PSUM)
    z = Z * (1/rowsum) fused into the PSUM->SBUF copy (DVE), DMA out fp32.

Accuracy: bf16 operands give 8.3e-3 rel err vs the fp32 reference on CPU
emulation (gate is 2e-2); fp32 PSUM accumulation throughout.
"""

from contextlib import ExitStack

import numpy as np

import concourse.bacc as bacc
import concourse.mybir as mybir
import concourse.tile as tile

F32 = mybir.dt.float32
BF16 = mybir.dt.bfloat16

B, S, D = 8, 2048, 1024
P = 128                    # SBUF partitions
TS = S // P                # 16 seq tiles
TD = D // P                # 8 d/e tiles
CH = 512                   # phase-A out chunk (matmul free dim, 1 PSUM bank)
NCH = S // CH              # 4
JC = 512                   # phase-B key chunk
NJC = S // JC              # 4
EC = 512                   # phase-B value-dim chunk
NEC = D // EC              # 2
SCALE = 1.0 / float(np.sqrt(D))


def build_program(repeats: int = 1, phases: str = "ab"):
    nc = bacc.Bacc("TRN2", target_bir_lowering=False, debug=False)

    xk = nc.dram_tensor("xk", [S, D], BF16, kind="ExternalInput").ap()
    xv = nc.dram_tensor("xv", [S, D], BF16, kind="ExternalInput").ap()
    xq = nc.dram_tensor("xq", [S, D], BF16, kind="ExternalInput").ap()
    wk = nc.dram_tensor("wk", [D, D], BF16, kind="ExternalInput").ap()
    wv = nc.dram_tensor("wv", [D, D], BF16, kind="ExternalInput").ap()
    wq = nc.dram_tensor("wq", [D, D], BF16, kind="ExternalInput").ap()
    z = nc.dram_tensor("z", [S, D], F32, kind="ExternalOutput").ap()

    with tile.TileContext(nc) as tc, ExitStack() as ctx:
        for rep in range(repeats):
            _one_pass(nc, tc, xk, xv, xq, wk, wv, wq, z, rep, phases)

    nc.compile()
    return nc


def _one_pass(nc, tc, xk, xv, xq, wk, wv, wq, z, rep, phases="ab"):
    with tc.tile_pool(name=f"res{rep}", bufs=1) as resident:
        kt = [resident.tile([P, S], BF16, tag=f"kt{e}", name=f"kt{e}") for e in range(TD)]
        qt = [resident.tile([P, S], BF16, tag=f"qt{e}", name=f"qt{e}") for e in range(TD)]
        vt = [resident.tile([P, D], BF16, tag=f"v{j}", name=f"v{j}") for j in range(TS)]

        # ---------------- Phase A: projections ----------------
        with (
            tc.tile_pool(name=f"xt{rep}", bufs=2) as xtp,
            tc.tile_pool(name=f"wp{rep}", bufs=2) as wp,
            tc.tile_pool(name=f"psA{rep}", bufs=8, space="PSUM") as psA,
        ):
            def load_xt(x_dram):
                # X^T via DMA XBAR: out[d-tile] [128 d, 2048 s], split across
                # both hwdge queues so the 8 transposes finish in ~7us.
                xt = xtp.tile([P, TD, S], BF16, tag="xt", name="xt")
                xsrc = x_dram.rearrange("s (dt p) -> s dt p", p=P)
                for d in range(TD):
                    nc.sync.dma_start(xt[:, d, :], xsrc[:, d, :], transpose=True)
                return xt

            def load_w(w_dram):
                w = wp.tile([P, TD, D], BF16, tag="w", name="w")
                wsrc = w_dram.rearrange("(dt p) e -> dt p e", p=P)
                for d in range(TD):
                    nc.sync.dma_start(w[:, d, :], wsrc[d])
                return w

            def proj_T(out_tiles, w, xt, startup=False):
                # out^T tile [128 e, CH seq] = accum_d w[d,e].T @ xT[d, chunk]
                if startup:
                    # First projection: the XBAR transposes of X are still
                    # landing, so run the c=0 chunk with the d-loop OUTER
                    # (8 concurrent PSUM chains) — each d round only needs
                    # xt[:, d] and overlaps the next transpose.
                    ps0 = [psA.tile([P, CH], F32, tag="psA", name="psA") for _ in range(TD)]
                    for d in range(TD):
                        for e in range(TD):
                            nc.tensor.matmul(
                                ps0[e][:],
                                w[:, d, e * P : (e + 1) * P],
                                xt[:, d, 0:CH],
                                start=(d == 0),
                                stop=(d == TD - 1),
                            )
                    for e in range(TD):
                        nc.vector.tensor_copy(out_tiles[e][:, 0:CH], ps0[e][:])
                c0 = 1 if startup else 0
                for e in range(TD):
                    for c in range(c0, NCH):
                        ps = psA.tile([P, CH], F32, tag="psA", name="psA")
                        for d in range(TD):
                            nc.tensor.matmul(
                                ps[:],
                                w[:, d, e * P : (e + 1) * P],
                                xt[:, d, c * CH : (c + 1) * CH],
                                start=(d == 0),
                                stop=(d == TD - 1),
                            )
                        nc.vector.tensor_copy(
                            out_tiles[e][:, c * CH : (c + 1) * CH], ps[:]
                        )

            def proj_V(w, xt):
                # V tile [128 seq, EC e] = accum_d xT[d, t].T @ w[d, chunk]
                for t in range(TS):
                    for ec in range(NEC):
                        ps = psA.tile([P, EC], F32, tag="psA", name="psA")
                        for d in range(TD):
                            nc.tensor.matmul(
                                ps[:],
                                xt[:, d, t * P : (t + 1) * P],
                                w[:, d, ec * EC : (ec + 1) * EC],
                                start=(d == 0),
                                stop=(d == TD - 1),
                            )
                        nc.vector.tensor_copy(
                            vt[t][:, ec * EC : (ec + 1) * EC], ps[:]
                        )

            xt_k = load_xt(xk)
            w_k = load_w(wk)
            xt_q = load_xt(xq)
            w_q = load_w(wq)
            proj_T(kt, w_k, xt_k, startup=True)
            proj_T(qt, w_q, xt_q)
            # xt_v/w_v reuse the k buffers; DMA starts as soon as the k
            # matmuls have consumed them, hiding under the q matmuls.
            xt_v = load_xt(xv)
            w_v = load_w(wv)
            proj_V(w_v, xt_v)

        if phases == "a":
            # A-only ablation: still produce z so the program has outputs.
            with tc.tile_pool(name=f"zoa{rep}", bufs=2) as zoa:
                for it in range(TS):
                    dummy = zoa.tile([P, D], F32, tag="dummy", name="dummy")
                    nc.vector.tensor_copy(dummy[:], vt[it][:])
                    nc.sync.dma_start(z[it * P : (it + 1) * P, :], dummy[:])
            return

        # ---------------- Phase B: attention ----------------
        with (
            tc.tile_pool(name=f"pb{rep}", bufs=2) as pbp,
            tc.tile_pool(name=f"pt{rep}", bufs=2) as ptp,
            tc.tile_pool(name=f"sc{rep}", bufs=2) as scp,
            tc.tile_pool(name=f"zo{rep}", bufs=2) as zop,
            tc.tile_pool(name=f"psS{rep}", bufs=4, space="PSUM") as psS,
            tc.tile_pool(name=f"psZ{rep}", bufs=3, space="PSUM") as psZ,
        ):
            def emit_qk(it):
                p_bf = pbp.tile([P, S], BF16, tag="p", name="p_bf")
                sums = scp.tile([P, NJC], F32, tag="sums", name="sums")
                ptall = ptp.tile([P, TS, P], BF16, tag="pt", name="ptall")
                for jc in range(NJC):
                    ps = psS.tile([P, JC], F32, tag="s", name="s_ps")
                    for e in range(TD):
                        nc.tensor.matmul(
                            ps[:],
                            qt[e][:, it * P : (it + 1) * P],
                            kt[e][:, jc * JC : (jc + 1) * JC],
                            start=(e == 0),
                            stop=(e == TD - 1),
                        )
                    nc.scalar.activation(
                        p_bf[:, jc * JC : (jc + 1) * JC],
                        ps[:],
                        mybir.ActivationFunctionType.Exp,
                        scale=SCALE,
                        accum_out=sums[:, jc : jc + 1],
                    )
                for j in range(TS):
                    nc.sync.dma_start(
                        ptall[:, j, :], p_bf[:, j * P : (j + 1) * P], transpose=True
                    )
                return sums, ptall

            def emit_pv(it, sums, ptall):
                s1 = scp.tile([P, 1], F32, tag="s1", name="s1")
                nc.vector.reduce_sum(s1[:], sums[:], axis=mybir.AxisListType.X)
                rec = scp.tile([P, 1], F32, tag="rec", name="rec")
                nc.vector.reciprocal(rec[:], s1[:])
                zo = zop.tile([P, D], F32, tag="zo", name="zo")
                for ec in range(NEC):
                    zp = psZ.tile([P, EC], F32, tag="z", name="z_ps")
                    for j in range(TS):
                        nc.tensor.matmul(
                            zp[:],
                            ptall[:, j, :],
                            vt[j][:, ec * EC : (ec + 1) * EC],
                            start=(j == 0),
                            stop=(j == TS - 1),
                        )
                    nc.vector.tensor_scalar_mul(
                        zo[:, ec * EC : (ec + 1) * EC], zp[:], rec[:]
                    )
                nc.sync.dma_start(z[it * P : (it + 1) * P, :], zo[:])

            prev = emit_qk(0)
            for it in range(TS):
                nxt = emit_qk(it + 1) if it + 1 < TS else None
                emit_pv(it, *prev)
                prev = nxt


_EXEC = None
_EXEC_BODY = None


def _build_exec(nc=None):
    """Compile the per-core program and wrap it in one jitted 8-core SPMD
    callable (shard_map over the 8 NeuronCores). Built once per process; the
    same callable serves correctness runs and timing loops."""
    import jax
    from jax.experimental.shard_map import shard_map
    from jax.sharding import Mesh, PartitionSpec

    from concourse import bass2jax

    if nc is None:
        nc = build_program()
    bass2jax.install_neuronx_cc_hook()

    partition_name = nc.partition_id_tensor.name if nc.partition_id_tensor else None
    in_names, out_names, out_avals, zero_outs = [], [], [], []
    for alloc in nc.m.functions[0].allocations:
        if not isinstance(alloc, mybir.MemoryLocationSet):
            continue
        name = alloc.memorylocations[0].name
        if alloc.kind == "ExternalInput":
            if name != partition_name:
                in_names.append(name)
        elif alloc.kind == "ExternalOutput":
            assert alloc.tensor_shape is not None and alloc.dtype is not None
            out_names.append(name)
            shape = tuple(alloc.tensor_shape)
            dtype = mybir.dt.np(alloc.dtype)
            out_avals.append(jax.core.ShapedArray(shape, dtype))
            zero_outs.append(np.zeros(shape, dtype))
    n_params = len(in_names)
    all_in_names = tuple(in_names) + tuple(out_names)
    if partition_name is not None:
        all_in_names = all_in_names + (partition_name,)

    def _body(*args):
        operands = list(args)
        if partition_name is not None:
            operands.append(bass2jax.partition_id_tensor())
        outs = bass2jax._bass_exec_p.bind(
            *operands,
            out_avals=tuple(out_avals),
            in_names=all_in_names,
            out_names=tuple(out_names),
            lowering_input_output_aliases=(),
            sim_require_finite=True,
            sim_require_nnan=True,
            nc=nc,
        )
        return tuple(outs)

    devices = jax.devices()[:B]
    assert len(devices) == B, f"need {B} cores, have {len(jax.devices())}"
    mesh = Mesh(np.asarray(devices), ("core",))
    n_outs = len(out_names)
    sharded_body = shard_map(
        _body,
        mesh=mesh,
        in_specs=(PartitionSpec("core"),) * (n_params + n_outs),
        out_specs=(PartitionSpec("core"),) * n_outs,
        check_rep=False,
    )
    global _EXEC_BODY
    _EXEC_BODY = sharded_body
    fn = jax.jit(sharded_body, keep_unused=True)
    return fn, mesh, in_names, out_names, zero_outs


def _get_exec():
    global _EXEC
    if _EXEC is None:
        _EXEC = _build_exec()
    return _EXEC


def _np_bf16():
    import ml_dtypes

    return ml_dtypes.bfloat16


def _concat_inputs(in_maps):
    """Per-core input dicts -> global concat arrays in executable order.
    Casts every input to the program's bf16 operand dtype."""
    fn, mesh, in_names, out_names, zero_outs = _get_exec()
    bf = _np_bf16()
    concat_in = [
        np.concatenate(
            [np.ascontiguousarray(in_maps[c][name], dtype=bf) for c in range(B)],
            axis=0,
        )
        for name in in_names
    ]
    concat_zeros = [
        np.zeros((B * z.shape[0], *z.shape[1:]), z.dtype) for z in zero_outs
    ]
    return concat_in + concat_zeros


def kernel(
    inputs_for_keys: np.ndarray,
    inputs_for_values: np.ndarray,
    inputs_for_queries: np.ndarray,
    W_K: np.ndarray,
    W_V: np.ndarray,
    W_Q: np.ndarray,
) -> np.ndarray:
    fn, mesh, in_names, out_names, zero_outs = _get_exec()
    in_maps = [
        {
            "xk": inputs_for_keys[b],
            "xv": inputs_for_values[b],
            "xq": inputs_for_queries[b],
            "wk": W_K,
            "wv": W_V,
            "wq": W_Q,
        }
        for b in range(B)
    ]
    out_arrs = fn(*_concat_inputs(in_maps))
    z_all = np.asarray(out_arrs[out_names.index("z")])
    return z_all.reshape(B, S, D)


if __name__ == "__main__":
    rng = np.random.default_rng(0)
    ins = {
        "inputs_for_keys": rng.standard_normal((B, S, D), dtype=np.float32),
        "inputs_for_values": rng.standard_normal((B, S, D), dtype=np.float32),
        "inputs_for_queries": rng.standard_normal((B, S, D), dtype=np.float32),
        "W_K": (rng.standard_normal((D, D)) * 0.05).astype(np.float32),
        "W_V": (rng.standard_normal((D, D)) * 0.05).astype(np.float32),
        "W_Q": (rng.standard_normal((D, D)) * 0.05).astype(np.float32),
    }
    out = kernel(**ins)
    print("out", out.shape, out.dtype)


# revision 7
# speedup vs baseline: 1.3482x; 1.3482x over previous
"""Single-head attention (B=8, S=2048, D_in=D_out=1024) on 8 Trainium2 NeuronCores.

Sharding: data-parallel over batch — core b computes batch element b end-to-end.
Weights (W_K/W_V/W_Q) are replicated to every core.

All matmul operands are bf16 (host-cast); PSUM accumulation is fp32. The PE
streams 1 cyc per output column for bf16 (same rate as fp32r), so matmul time
is unchanged vs the fp32r baseline, but bf16 unlocks the DMA XBAR transpose
(16x128 tiles @ 14ns, runs on the DMA engines) which removes ALL transposes
from the PE:
  - X^T [d, s] tiles are produced by DMA-transposing X straight out of DRAM.
  - P^T tiles are produced by SBUF->SBUF DMA transpose of the softmax rows.
The fp32r baseline spent ~70us of PE time on 640 identity-matmul transposes;
here the PE does only the 1792 "real" matmuls (~382us at 2.4 GHz).

Per-core program:
  Phase A (projections; contraction d on SBUF partitions via XBAR X^T):
    K^T = accum_d W_k[d]^T @ X_k^T[d]  -> 8 tiles [128 e, 2048 j]   (SBUF, bf16)
    Q^T = accum_d W_q[d]^T @ X_q^T[d]  -> 8 tiles [128 e, 2048 i]   (SBUF, bf16)
    V   = accum_d X_v^T[d]^T @ W_v[d]  -> 16 tiles [128 j, 1024 e]  (SBUF, bf16)
    (Q^T is SBUF-resident in bf16 — no DRAM scratch round-trip.)
  Phase B (attention, per 128-query tile, software-pipelined so the PE
  stream is [qk(0)][qk(1)][pv(0)][qk(2)][pv(1)]... and exp/transpose latency
  of tile it hides under qk(it+1)):
    S chunk [128 i, 512 j] = accum_e qt[e][:,it].T @ kt[e][:,chunk]  (PSUM)
    P chunk = exp(S/32) on ACT (bf16 out) with fused row-sum accumulation.
      No max subtraction: scores are O(+-17), exp stays inside fp32/bf16
      range, softmax is shift-invariant.
    P^T tiles [128 j, 128 i] via DMA XBAR (112ns each, off the PE)
    Z [128 i, 1024 e] = accum_j pt[j].T @ v[j]                       (PSUM)
    z = Z * (1/rowsum) fused into the PSUM->SBUF copy (DVE), DMA out fp32.

Accuracy: bf16 operands give 8.3e-3 rel err vs the fp32 reference on CPU
emulation (gate is 2e-2); fp32 PSUM accumulation throughout.
"""

from contextlib import ExitStack

import numpy as np

import concourse.bacc as bacc
import concourse.mybir as mybir
import concourse.tile as tile

F32 = mybir.dt.float32
BF16 = mybir.dt.bfloat16

B, S, D = 8, 2048, 1024
P = 128                    # SBUF partitions
TS = S // P                # 16 seq tiles
TD = D // P                # 8 d/e tiles
CH = 512                   # phase-A out chunk (matmul free dim, 1 PSUM bank)
NCH = S // CH              # 4
JC = 512                   # phase-B key chunk
NJC = S // JC              # 4
EC = 512                   # phase-B value-dim chunk
NEC = D // EC              # 2
SCALE = 1.0 / float(np.sqrt(D))


def build_program(repeats: int = 1, phases: str = "ab"):
    nc = bacc.Bacc("TRN2", target_bir_lowering=False, debug=False)

    xk = nc.dram_tensor("xk", [S, D], BF16, kind="ExternalInput").ap()
    xv = nc.dram_tensor("xv", [S, D], BF16, kind="ExternalInput").ap()
    xq = nc.dram_tensor("xq", [S, D], BF16, kind="ExternalInput").ap()
    wk = nc.dram_tensor("wk", [D, D], BF16, kind="ExternalInput").ap()
    wv = nc.dram_tensor("wv", [D, D], BF16, kind="ExternalInput").ap()
    wq = nc.dram_tensor("wq", [D, D], BF16, kind="ExternalInput").ap()
    z = nc.dram_tensor("z", [S, D], F32, kind="ExternalOutput").ap()

    with tile.TileContext(nc) as tc, ExitStack() as ctx:
        for rep in range(repeats):
            _one_pass(nc, tc, xk, xv, xq, wk, wv, wq, z, rep, phases)

    nc.compile()
    return nc


def _one_pass(nc, tc, xk, xv, xq, wk, wv, wq, z, rep, phases="ab"):
    with tc.tile_pool(name=f"res{rep}", bufs=1) as resident:
        kt = [resident.tile([P, S], BF16, tag=f"kt{e}", name=f"kt{e}") for e in range(TD)]
        qt = [resident.tile([P, S], BF16, tag=f"qt{e}", name=f"qt{e}") for e in range(TD)]
        vt = [resident.tile([P, D], BF16, tag=f"v{j}", name=f"v{j}") for j in range(TS)]

        # ---------------- Phase A: projections ----------------
        with (
            tc.tile_pool(name=f"xt{rep}", bufs=2) as xtp,
            tc.tile_pool(name=f"wp{rep}", bufs=2) as wp,
            tc.tile_pool(name=f"psA{rep}", bufs=8, space="PSUM") as psA,
        ):
            def load_xt(x_dram, split=False):
                # X^T via DMA XBAR. All DMAs live on the single SP queue (the
                # XBAR corrupts data when ANY other DMA runs concurrently),
                # so batching matters: one instruction transposes the whole
                # [2048, 1024] input into the 3D tile ([:, d, :] = X^T tile d).
                # split=True issues per-d transposes instead so the first
                # projection can start after tile d=0 lands (~2us).
                xt = xtp.tile([P, TD, S], BF16, tag="xt", name="xt")
                if split:
                    xsrc = x_dram.rearrange("s (dt p) -> s dt p", p=P)
                    for d in range(TD):
                        nc.sync.dma_start(xt[:, d, :], xsrc[:, d, :], transpose=True)
                else:
                    nc.sync.dma_start(xt[:], x_dram, transpose=True)
                return xt

            def load_w(w_dram):
                # one 3D-AP DMA: w[p, dt, e] = w_dram[dt*128 + p, e]
                w = wp.tile([P, TD, D], BF16, tag="w", name="w")
                nc.sync.dma_start(w[:], w_dram.rearrange("(dt p) e -> p dt e", p=P))
                return w

            def proj_T(out_tiles, w, xt, startup=False):
                # out^T tile [128 e, CH seq] = accum_d w[d,e].T @ xT[d, chunk]
                if startup:
                    # First projection: the XBAR transposes of X are still
                    # landing, so run the c=0 chunk with the d-loop OUTER
                    # (8 concurrent PSUM chains) — each d round only needs
                    # xt[:, d] and overlaps the next transpose.
                    ps0 = [psA.tile([P, CH], F32, tag="psA", name="psA") for _ in range(TD)]
                    for d in range(TD):
                        for e in range(TD):
                            nc.tensor.matmul(
                                ps0[e][:],
                                w[:, d, e * P : (e + 1) * P],
                                xt[:, d, 0:CH],
                                start=(d == 0),
                                stop=(d == TD - 1),
                            )
                    for e in range(TD):
                        nc.vector.tensor_copy(out_tiles[e][:, 0:CH], ps0[e][:])
                c0 = 1 if startup else 0
                for e in range(TD):
                    for c in range(c0, NCH):
                        ps = psA.tile([P, CH], F32, tag="psA", name="psA")
                        for d in range(TD):
                            nc.tensor.matmul(
                                ps[:],
                                w[:, d, e * P : (e + 1) * P],
                                xt[:, d, c * CH : (c + 1) * CH],
                                start=(d == 0),
                                stop=(d == TD - 1),
                            )
                        nc.vector.tensor_copy(
                            out_tiles[e][:, c * CH : (c + 1) * CH], ps[:]
                        )

            def proj_V(w, xt):
                # V tile [128 seq, EC e] = accum_d xT[d, t].T @ w[d, chunk]
                for t in range(TS):
                    for ec in range(NEC):
                        ps = psA.tile([P, EC], F32, tag="psA", name="psA")
                        for d in range(TD):
                            nc.tensor.matmul(
                                ps[:],
                                xt[:, d, t * P : (t + 1) * P],
                                w[:, d, ec * EC : (ec + 1) * EC],
                                start=(d == 0),
                                stop=(d == TD - 1),
                            )
                        nc.vector.tensor_copy(
                            vt[t][:, ec * EC : (ec + 1) * EC], ps[:]
                        )

            w_k = load_w(wk)
            xt_k = load_xt(xk, split=True)
            w_q = load_w(wq)
            xt_q = load_xt(xq)
            proj_T(kt, w_k, xt_k, startup=True)
            proj_T(qt, w_q, xt_q)
            # xt_v/w_v reuse the k buffers; DMA starts as soon as the k
            # matmuls have consumed them, hiding under the q matmuls.
            w_v = load_w(wv)
            xt_v = load_xt(xv)
            proj_V(w_v, xt_v)

        if phases == "a":
            # A-only ablation: still produce z so the program has outputs.
            with tc.tile_pool(name=f"zoa{rep}", bufs=2) as zoa:
                for it in range(TS):
                    dummy = zoa.tile([P, D], F32, tag="dummy", name="dummy")
                    nc.vector.tensor_copy(dummy[:], vt[it][:])
                    nc.sync.dma_start(z[it * P : (it + 1) * P, :], dummy[:])
            return

        # ---------------- Phase B: attention ----------------
        with (
            tc.tile_pool(name=f"pb{rep}", bufs=2) as pbp,
            tc.tile_pool(name=f"pt{rep}", bufs=2) as ptp,
            tc.tile_pool(name=f"sc{rep}", bufs=2) as scp,
            tc.tile_pool(name=f"zo{rep}", bufs=2) as zop,
            tc.tile_pool(name=f"psS{rep}", bufs=4, space="PSUM") as psS,
            tc.tile_pool(name=f"psZ{rep}", bufs=3, space="PSUM") as psZ,
        ):
            def emit_qk(it):
                p_bf = pbp.tile([P, S], BF16, tag="p", name="p_bf")
                sums = scp.tile([P, NJC], F32, tag="sums", name="sums")
                ptall = ptp.tile([P, TS, P], BF16, tag="pt", name="ptall")
                for jc in range(NJC):
                    ps = psS.tile([P, JC], F32, tag="s", name="s_ps")
                    for e in range(TD):
                        nc.tensor.matmul(
                            ps[:],
                            qt[e][:, it * P : (it + 1) * P],
                            kt[e][:, jc * JC : (jc + 1) * JC],
                            start=(e == 0),
                            stop=(e == TD - 1),
                        )
                    nc.scalar.activation(
                        p_bf[:, jc * JC : (jc + 1) * JC],
                        ps[:],
                        mybir.ActivationFunctionType.Exp,
                        scale=SCALE,
                        accum_out=sums[:, jc : jc + 1],
                    )
                # one XBAR instruction transposes all 16 P^T tiles
                nc.sync.dma_start(ptall[:], p_bf[:], transpose=True)
                return sums, ptall

            def emit_pv(it, sums, ptall):
                s1 = scp.tile([P, 1], F32, tag="s1", name="s1")
                nc.vector.reduce_sum(s1[:], sums[:], axis=mybir.AxisListType.X)
                rec = scp.tile([P, 1], F32, tag="rec", name="rec")
                nc.vector.reciprocal(rec[:], s1[:])
                zo = zop.tile([P, D], F32, tag="zo", name="zo")
                for ec in range(NEC):
                    zp = psZ.tile([P, EC], F32, tag="z", name="z_ps")
                    for j in range(TS):
                        nc.tensor.matmul(
                            zp[:],
                            ptall[:, j, :],
                            vt[j][:, ec * EC : (ec + 1) * EC],
                            start=(j == 0),
                            stop=(j == TS - 1),
                        )
                    nc.vector.tensor_scalar_mul(
                        zo[:, ec * EC : (ec + 1) * EC], zp[:], rec[:]
                    )
                nc.sync.dma_start(z[it * P : (it + 1) * P, :], zo[:])

            prev = emit_qk(0)
            for it in range(TS):
                nxt = emit_qk(it + 1) if it + 1 < TS else None
                emit_pv(it, *prev)
                prev = nxt


_EXEC = None
_EXEC_BODY = None


def _build_exec(nc=None):
    """Compile the per-core program and wrap it in one jitted 8-core SPMD
    callable (shard_map over the 8 NeuronCores). Built once per process; the
    same callable serves correctness runs and timing loops."""
    import jax
    from jax.experimental.shard_map import shard_map
    from jax.sharding import Mesh, PartitionSpec

    from concourse import bass2jax

    if nc is None:
        nc = build_program()
    bass2jax.install_neuronx_cc_hook()

    partition_name = nc.partition_id_tensor.name if nc.partition_id_tensor else None
    in_names, out_names, out_avals, zero_outs = [], [], [], []
    for alloc in nc.m.functions[0].allocations:
        if not isinstance(alloc, mybir.MemoryLocationSet):
            continue
        name = alloc.memorylocations[0].name
        if alloc.kind == "ExternalInput":
            if name != partition_name:
                in_names.append(name)
        elif alloc.kind == "ExternalOutput":
            assert alloc.tensor_shape is not None and alloc.dtype is not None
            out_names.append(name)
            shape = tuple(alloc.tensor_shape)
            dtype = mybir.dt.np(alloc.dtype)
            out_avals.append(jax.core.ShapedArray(shape, dtype))
            zero_outs.append(np.zeros(shape, dtype))
    n_params = len(in_names)
    all_in_names = tuple(in_names) + tuple(out_names)
    if partition_name is not None:
        all_in_names = all_in_names + (partition_name,)

    def _body(*args):
        operands = list(args)
        if partition_name is not None:
            operands.append(bass2jax.partition_id_tensor())
        outs = bass2jax._bass_exec_p.bind(
            *operands,
            out_avals=tuple(out_avals),
            in_names=all_in_names,
            out_names=tuple(out_names),
            lowering_input_output_aliases=(),
            sim_require_finite=True,
            sim_require_nnan=True,
            nc=nc,
        )
        return tuple(outs)

    devices = jax.devices()[:B]
    assert len(devices) == B, f"need {B} cores, have {len(jax.devices())}"
    mesh = Mesh(np.asarray(devices), ("core",))
    n_outs = len(out_names)
    sharded_body = shard_map(
        _body,
        mesh=mesh,
        in_specs=(PartitionSpec("core"),) * (n_params + n_outs),
        out_specs=(PartitionSpec("core"),) * n_outs,
        check_rep=False,
    )
    global _EXEC_BODY
    _EXEC_BODY = sharded_body
    fn = jax.jit(sharded_body, keep_unused=True)
    return fn, mesh, in_names, out_names, zero_outs


def _get_exec():
    global _EXEC
    if _EXEC is None:
        _EXEC = _build_exec()
    return _EXEC


def _np_bf16():
    import ml_dtypes

    return ml_dtypes.bfloat16


def _concat_inputs(in_maps):
    """Per-core input dicts -> global concat arrays in executable order.
    Casts every input to the program's bf16 operand dtype."""
    fn, mesh, in_names, out_names, zero_outs = _get_exec()
    bf = _np_bf16()
    concat_in = [
        np.concatenate(
            [np.ascontiguousarray(in_maps[c][name], dtype=bf) for c in range(B)],
            axis=0,
        )
        for name in in_names
    ]
    concat_zeros = [
        np.zeros((B * z.shape[0], *z.shape[1:]), z.dtype) for z in zero_outs
    ]
    return concat_in + concat_zeros


def kernel(
    inputs_for_keys: np.ndarray,
    inputs_for_values: np.ndarray,
    inputs_for_queries: np.ndarray,
    W_K: np.ndarray,
    W_V: np.ndarray,
    W_Q: np.ndarray,
) -> np.ndarray:
    fn, mesh, in_names, out_names, zero_outs = _get_exec()
    in_maps = [
        {
            "xk": inputs_for_keys[b],
            "xv": inputs_for_values[b],
            "xq": inputs_for_queries[b],
            "wk": W_K,
            "wv": W_V,
            "wq": W_Q,
        }
        for b in range(B)
    ]
    out_arrs = fn(*_concat_inputs(in_maps))
    z_all = np.asarray(out_arrs[out_names.index("z")])
    return z_all.reshape(B, S, D)


if __name__ == "__main__":
    rng = np.random.default_rng(0)
    ins = {
        "inputs_for_keys": rng.standard_normal((B, S, D), dtype=np.float32),
        "inputs_for_values": rng.standard_normal((B, S, D), dtype=np.float32),
        "inputs_for_queries": rng.standard_normal((B, S, D), dtype=np.float32),
        "W_K": (rng.standard_normal((D, D)) * 0.05).astype(np.float32),
        "W_V": (rng.standard_normal((D, D)) * 0.05).astype(np.float32),
        "W_Q": (rng.standard_normal((D, D)) * 0.05).astype(np.float32),
    }
    out = kernel(**ins)
    print("out", out.shape, out.dtype)


# revision 23
# speedup vs baseline: 2.1837x; 1.6197x over previous
"""Single-head attention (B=8, S=2048, D_in=D_out=1024) on 8 Trainium2 NeuronCores.

Sharding: data-parallel over batch — core b computes batch element b end-to-end.
Weights (W_K/W_V/W_Q) are replicated to every core.

All matmul operands are bf16 (host-cast); PSUM accumulation is fp32. The PE
streams 1 cyc per output column for bf16 (same rate as fp32r), so matmul time
is unchanged vs the fp32r baseline, but bf16 unlocks the DMA XBAR transpose
(16x128 tiles @ 14ns, runs on the DMA engines) which removes ALL transposes
from the PE:
  - X^T [d, s] tiles are produced by DMA-transposing X straight out of DRAM.
  - P^T tiles are produced by SBUF->SBUF DMA transpose of the softmax rows.
The fp32r baseline spent ~70us of PE time on 640 identity-matmul transposes;
here the PE does only the 1792 "real" matmuls (~382us at 2.4 GHz).

HARDWARE CONSTRAINT (measured, invisible in CoreSim): the XBAR transpose
corrupts data if ANY other DMA runs concurrently on the other hwdge queue
(~1% 16-column shifts for xbar||xbar, rare single elements for xbar||copy).
So every DMA in the program is issued on the single SP (nc.sync) queue, and
DMA count is minimized by batching: one XBAR instruction transposes a whole
input ([128, 8, 2048] 3D out tile), one 3D-AP DMA loads a whole weight
matrix; 38 DMAs total per pass (per-DMA dispatch is ~1.8us on this queue —
an earlier 320-DMA version lost ~170us to it).

Per-core program:
  Phase A (projections; contraction d on SBUF partitions via XBAR X^T):
    K^T = accum_d W_k[d]^T @ X_k^T[d]  -> 8 tiles [128 e, 2048 j]   (SBUF, bf16)
    Q^T = accum_d W_q[d]^T @ X_q^T[d]  -> 8 tiles [128 e, 2048 i]   (SBUF, bf16)
    V   = accum_d X_v^T[d]^T @ W_v[d]  -> 16 tiles [128 j, 1024 e]  (SBUF, bf16)
    (Q^T is SBUF-resident in bf16 — no DRAM scratch round-trip. The first
    projection loads X^T per-d with the d-loop outermost across 8 PSUM
    chains, so the PE starts ~2us in instead of waiting ~15us for the
    full transpose.)
  Phase B (attention, per 128-query tile it, software-pipelined at depth 4:
  PE stream is qk0..qk3 pv0 qk4 pv1 ... so pv(it) starts four 6.8us
  qk-windows after qk(it), hiding the exp + XBAR-transpose latency that
  produces ptall(it) even under DMA contention — pipelining this was
  worth ~150us on HW):
    S chunk [128 i, 512 j] = accum_e qt[e][:,it].T @ kt[e][:,chunk]  (PSUM)
    P chunk = exp(S/32) on ACT (bf16 out) with fused row-sum accumulation.
      No max subtraction: scores are O(+-17), exp stays inside fp32/bf16
      range, softmax is shift-invariant.
    P^T tiles [128 j, 128 i] via one XBAR instruction (1.8us, off the PE)
    Z [128 i, 1024 e] = accum_j pt[j].T @ v[j]                       (PSUM)
    z = Z * (1/rowsum) fused into the PSUM->SBUF copy (DVE), DMA out bf16
    (host upcasts to fp32 — halves the largest HBM write).

Accuracy: bf16 operands + bf16 output give 8.75e-3 rel err vs the fp32
reference on HW (gate is 2e-2); fp32 PSUM accumulation throughout.

Measured on 8x trn2 NeuronCores (slope method, overhead-cancelled):
~298-320us per full forward on a quiet device (best 297.7us with the
deferred-z rep overlap; 6 quiet-window runs in 298-317us), ~400-460us when
the shared device is contended. The fp32r baseline measured 581-610us on
the same setup. TimelineSim predicts 408us single-pass / 395us marginal
rep; HW beats the sim's 1 cyc/output-column matmul model, so on a quiet
device the PE is essentially 100% busy and the kernel is at the hardware's
real matmul-streaming roofline for this dtype.
"""

from contextlib import ExitStack

import numpy as np

import concourse.bacc as bacc
import concourse.mybir as mybir
import concourse.tile as tile

F32 = mybir.dt.float32
BF16 = mybir.dt.bfloat16

B, S, D = 8, 2048, 1024
P = 128                    # SBUF partitions
TS = S // P                # 16 seq tiles
TD = D // P                # 8 d/e tiles
CH = 512                   # phase-A out chunk (matmul free dim, 1 PSUM bank)
NCH = S // CH              # 4
JC = 512                   # phase-B key chunk
NJC = S // JC              # 4
EC = 512                   # phase-B value-dim chunk
NEC = D // EC              # 2
SCALE = 1.0 / float(np.sqrt(D))


def build_program(repeats: int = 1, phases: str = "ab"):
    nc = bacc.Bacc("TRN2", target_bir_lowering=False, debug=False)

    xk = nc.dram_tensor("xk", [S, D], BF16, kind="ExternalInput").ap()
    xv = nc.dram_tensor("xv", [S, D], BF16, kind="ExternalInput").ap()
    xq = nc.dram_tensor("xq", [S, D], BF16, kind="ExternalInput").ap()
    wk = nc.dram_tensor("wk", [D, D], BF16, kind="ExternalInput").ap()
    wv = nc.dram_tensor("wv", [D, D], BF16, kind="ExternalInput").ap()
    wq = nc.dram_tensor("wq", [D, D], BF16, kind="ExternalInput").ap()
    # z is written bf16 (halves the largest HBM write: 8MB -> 4MB per core
    # per pass) and upcast to fp32 on the host; adds ~0.2% rounding to the
    # ~0.86% bf16 pipeline error, well inside the 2e-2 gate.
    z = nc.dram_tensor("z", [S, D], BF16, kind="ExternalOutput").ap()

    with tile.TileContext(nc) as tc, tc.tile_pool(name="zop", bufs=3) as zop:
        # zo lives in a program-lifetime pool so the last two z-output DMAs
        # of rep r can be EMITTED inside rep r+1's phase A: the next rep's
        # weight/X^T loads then sit ahead of them on the SP queue and
        # prefetch during rep r's tail compute, removing the ~10us
        # rep-boundary PE bubble of the repeated timing program.
        deferred = []
        for rep in range(repeats):
            deferred = _one_pass(
                nc, tc, xk, xv, xq, wk, wv, wq, z, rep, phases, zop, deferred
            )
        for out_ap, zo in deferred:
            nc.sync.dma_start(out_ap, zo[:])

    nc.compile()
    return nc


def _one_pass(nc, tc, xk, xv, xq, wk, wv, wq, z, rep, phases, zop, deferred_z):
    with tc.tile_pool(name=f"res{rep}", bufs=1) as resident:
        gt_all = resident.tile([P, TD, S], BF16, tag="gt", name="gt_all")
        xq_all = resident.tile([P, TD, S], BF16, tag="xq", name="xq_all")
        vt = [resident.tile([P, D], BF16, tag=f"v{j}", name=f"v{j}") for j in range(TS)]

        # ------------- Phase A: score-fused projections -------------
        # S = (Xq Wq)(Xk Wk)^T reassociated as Xq @ (A @ Xk^T) with
        # A = Wq Wk^T: A costs 128 matmuls and G = A Xk^T costs 256,
        # replacing the 512-matmul Q/K projections (-128 PE instructions).
        # The W^T XBARs are split per e-tile and A's first half accumulates
        # with the e-loop OUTER across 8 PSUM chains, so the PE starts ~3us
        # in and stays fed (an unsplit version stalled ~15us at every rep
        # boundary waiting for the whole W^T transpose).
        with (
            tc.tile_pool(name=f"at{rep}", bufs=1) as atp,
            tc.tile_pool(name=f"psA{rep}", bufs=8, space="PSUM") as psA,
        ):
            at = atp.tile([P, TD, D], BF16, tag="at", name="at")
            with tc.tile_pool(name=f"wt{rep}", bufs=1) as wtp:
                wqT = wtp.tile([P, TD, D], BF16, tag="wqT", name="wqT")
                wkT = wtp.tile([P, TD, D], BF16, tag="wkT", name="wkT")
                for e in range(TD):
                    nc.sync.dma_start(
                        wkT[:, e, :], wk[:, e * P : (e + 1) * P], transpose=True
                    )
                    nc.sync.dma_start(
                        wqT[:, e, :], wq[:, e * P : (e + 1) * P], transpose=True
                    )
                # previous rep's tail z-outputs: enqueued after the wT loads
                # so those prefetch during the prior rep's tail compute.
                for out_ap, zo_prev in deferred_z:
                    nc.sync.dma_start(out_ap, zo_prev[:])
                # A^T[d', d] = Wk Wq^T: chunk ch=0 with e OUTER (8 chains)
                ps0 = [psA.tile([P, EC], F32, tag="psA", name="psA") for _ in range(TD)]
                for e in range(TD):
                    for dpt in range(TD):
                        nc.tensor.matmul(
                            ps0[dpt][:],
                            wkT[:, e, dpt * P : (dpt + 1) * P],
                            wqT[:, e, 0:EC],
                            start=(e == 0),
                            stop=(e == TD - 1),
                        )
                for dpt in range(TD):
                    nc.vector.tensor_copy(at[:, dpt, 0:EC], ps0[dpt][:])
                for dpt in range(TD):
                    ps = psA.tile([P, EC], F32, tag="psA", name="psA")
                    for e in range(TD):
                        nc.tensor.matmul(
                            ps[:],
                            wkT[:, e, dpt * P : (dpt + 1) * P],
                            wqT[:, e, EC : 2 * EC],
                            start=(e == 0),
                            stop=(e == TD - 1),
                        )
                    nc.vector.tensor_copy(at[:, dpt, EC : 2 * EC], ps[:])
            with (
                tc.tile_pool(name=f"xt{rep}", bufs=2) as xtp,
                tc.tile_pool(name=f"wp{rep}", bufs=1) as wp,
            ):
                xt_k = xtp.tile([P, TD, S], BF16, tag="xt", name="xt")
                nc.sync.dma_start(xt_k[:], xk, transpose=True)
                w_v = wp.tile([P, TD, D], BF16, tag="w", name="w")
                nc.sync.dma_start(w_v[:], wv.rearrange("(dt p) e -> p dt e", p=P))
                nc.sync.dma_start(xq_all[:], xq, transpose=True)
                xt_v = xtp.tile([P, TD, S], BF16, tag="xt", name="xt")
                nc.sync.dma_start(xt_v[:], xv, transpose=True)
                # G tile [128 d, JC j] = accum_d' A^T[d',d].T @ Xk^T[d', jc]
                for dt in range(TD):
                    for jc in range(NJC):
                        ps = psA.tile([P, JC], F32, tag="psA", name="psA")
                        for dp in range(TD):
                            nc.tensor.matmul(
                                ps[:],
                                at[:, dp, dt * P : (dt + 1) * P],
                                xt_k[:, dp, jc * JC : (jc + 1) * JC],
                                start=(dp == 0),
                                stop=(dp == TD - 1),
                            )
                        nc.vector.tensor_copy(
                            gt_all[:, dt, jc * JC : (jc + 1) * JC], ps[:]
                        )
                # V tile [128 seq, EC e] = accum_d xT[d, t].T @ w[d, chunk]
                for t in range(TS):
                    for ec in range(NEC):
                        ps = psA.tile([P, EC], F32, tag="psA", name="psA")
                        for d in range(TD):
                            nc.tensor.matmul(
                                ps[:],
                                xt_v[:, d, t * P : (t + 1) * P],
                                w_v[:, d, ec * EC : (ec + 1) * EC],
                                start=(d == 0),
                                stop=(d == TD - 1),
                            )
                        nc.vector.tensor_copy(
                            vt[t][:, ec * EC : (ec + 1) * EC], ps[:]
                        )

        if phases == "a":
            # A-only ablation: still produce z so the program has outputs.
            with tc.tile_pool(name=f"zoa{rep}", bufs=2) as zoa:
                for it in range(TS):
                    dummy = zoa.tile([P, D], BF16, tag="dummy", name="dummy")
                    nc.vector.tensor_copy(dummy[:], vt[it][:])
                    nc.sync.dma_start(z[it * P : (it + 1) * P, :], dummy[:])
            return []

        # ---------------- Phase B: attention ----------------
        with (
            tc.tile_pool(name=f"pb{rep}", bufs=5) as pbp,
            tc.tile_pool(name=f"pt{rep}", bufs=5) as ptp,
            tc.tile_pool(name=f"sc{rep}", bufs=5) as scp,
            tc.tile_pool(name=f"psS{rep}", bufs=4, space="PSUM") as psS,
            tc.tile_pool(name=f"psZ{rep}", bufs=3, space="PSUM") as psZ,
        ):
            def emit_qk(it):
                p_bf = pbp.tile([P, S], BF16, tag="p", name="p_bf")
                sums = scp.tile([P, NJC], F32, tag="sums", name="sums")
                ptall = ptp.tile([P, TS, P], BF16, tag="pt", name="ptall")
                for jc in range(NJC):
                    ps = psS.tile([P, JC], F32, tag="s", name="s_ps")
                    for e in range(TD):
                        nc.tensor.matmul(
                            ps[:],
                            xq_all[:, e, it * P : (it + 1) * P],
                            gt_all[:, e, jc * JC : (jc + 1) * JC],
                            start=(e == 0),
                            stop=(e == TD - 1),
                        )
                    nc.scalar.activation(
                        p_bf[:, jc * JC : (jc + 1) * JC],
                        ps[:],
                        mybir.ActivationFunctionType.Exp,
                        scale=SCALE,
                        accum_out=sums[:, jc : jc + 1],
                    )
                # one XBAR instruction transposes all 16 P^T tiles
                nc.sync.dma_start(ptall[:], p_bf[:], transpose=True)
                return sums, ptall

            def emit_pv(it, sums, ptall):
                s1 = scp.tile([P, 1], F32, tag="s1", name="s1")
                nc.vector.reduce_sum(s1[:], sums[:], axis=mybir.AxisListType.X)
                rec = scp.tile([P, 1], F32, tag="rec", name="rec")
                nc.vector.reciprocal(rec[:], s1[:])
                zo = zop.tile([P, D], BF16, tag="zo", name="zo")
                for ec in range(NEC):
                    zp = psZ.tile([P, EC], F32, tag="z", name="z_ps")
                    for j in range(TS):
                        nc.tensor.matmul(
                            zp[:],
                            ptall[:, j, :],
                            vt[j][:, ec * EC : (ec + 1) * EC],
                            start=(j == 0),
                            stop=(j == TS - 1),
                        )
                    nc.vector.tensor_scalar_mul(
                        zo[:, ec * EC : (ec + 1) * EC], zp[:], rec[:]
                    )
                return (z[it * P : (it + 1) * P, :], zo)

            # depth-4 pipeline: PE stream is qk0..qk3 pv0 qk4 pv1 ... so
            # pv(it) starts four 6.8us qk-windows after qk(it) — covers the
            # exp+XBAR latency producing ptall(it) with ~27us of slack, since
            # that XBAR is the only DMA on the phase-B critical path and the
            # shared DMA engines can be contended by other tenants. Depth 2
            # sufficed on a quiet device; extra depth costs only SBUF.
            from collections import deque
            q = deque([emit_qk(0), emit_qk(1), emit_qk(2), emit_qk(3)])
            new_deferred = []
            for it in range(TS):
                if it + 4 < TS:
                    q.append(emit_qk(it + 4))
                pair = emit_pv(it, *q.popleft())
                if it < TS - 2:
                    nc.sync.dma_start(pair[0], pair[1][:])
                else:
                    new_deferred.append(pair)
            return new_deferred


_EXEC = None
_EXEC_BODY = None


def _build_exec(nc=None):
    """Compile the per-core program and wrap it in one jitted 8-core SPMD
    callable (shard_map over the 8 NeuronCores). Built once per process; the
    same callable serves correctness runs and timing loops."""
    import jax
    from jax.experimental.shard_map import shard_map
    from jax.sharding import Mesh, PartitionSpec

    from concourse import bass2jax

    if nc is None:
        nc = build_program()
    bass2jax.install_neuronx_cc_hook()

    partition_name = nc.partition_id_tensor.name if nc.partition_id_tensor else None
    in_names, out_names, out_avals, zero_outs = [], [], [], []
    for alloc in nc.m.functions[0].allocations:
        if not isinstance(alloc, mybir.MemoryLocationSet):
            continue
        name = alloc.memorylocations[0].name
        if alloc.kind == "ExternalInput":
            if name != partition_name:
                in_names.append(name)
        elif alloc.kind == "ExternalOutput":
            assert alloc.tensor_shape is not None and alloc.dtype is not None
            out_names.append(name)
            shape = tuple(alloc.tensor_shape)
            dtype = mybir.dt.np(alloc.dtype)
            out_avals.append(jax.core.ShapedArray(shape, dtype))
            zero_outs.append(np.zeros(shape, dtype))
    n_params = len(in_names)
    all_in_names = tuple(in_names) + tuple(out_names)
    if partition_name is not None:
        all_in_names = all_in_names + (partition_name,)

    def _body(*args):
        operands = list(args)
        if partition_name is not None:
            operands.append(bass2jax.partition_id_tensor())
        outs = bass2jax._bass_exec_p.bind(
            *operands,
            out_avals=tuple(out_avals),
            in_names=all_in_names,
            out_names=tuple(out_names),
            lowering_input_output_aliases=(),
            sim_require_finite=True,
            sim_require_nnan=True,
            nc=nc,
        )
        return tuple(outs)

    devices = jax.devices()[:B]
    assert len(devices) == B, f"need {B} cores, have {len(jax.devices())}"
    mesh = Mesh(np.asarray(devices), ("core",))
    n_outs = len(out_names)
    sharded_body = shard_map(
        _body,
        mesh=mesh,
        in_specs=(PartitionSpec("core"),) * (n_params + n_outs),
        out_specs=(PartitionSpec("core"),) * n_outs,
        check_rep=False,
    )
    global _EXEC_BODY
    _EXEC_BODY = sharded_body
    fn = jax.jit(sharded_body, keep_unused=True)
    return fn, mesh, in_names, out_names, zero_outs


def _get_exec():
    global _EXEC
    if _EXEC is None:
        _EXEC = _build_exec()
    return _EXEC


def _np_bf16():
    import ml_dtypes

    return ml_dtypes.bfloat16


def _concat_inputs(in_maps):
    """Per-core input dicts -> global concat arrays in executable order.
    Casts every input to the program's bf16 operand dtype."""
    fn, mesh, in_names, out_names, zero_outs = _get_exec()
    bf = _np_bf16()
    concat_in = [
        np.concatenate(
            [np.ascontiguousarray(in_maps[c][name], dtype=bf) for c in range(B)],
            axis=0,
        )
        for name in in_names
    ]
    concat_zeros = [
        np.zeros((B * z.shape[0], *z.shape[1:]), z.dtype) for z in zero_outs
    ]
    return concat_in + concat_zeros


def kernel(
    inputs_for_keys: np.ndarray,
    inputs_for_values: np.ndarray,
    inputs_for_queries: np.ndarray,
    W_K: np.ndarray,
    W_V: np.ndarray,
    W_Q: np.ndarray,
) -> np.ndarray:
    fn, mesh, in_names, out_names, zero_outs = _get_exec()
    in_maps = [
        {
            "xk": inputs_for_keys[b],
            "xv": inputs_for_values[b],
            "xq": inputs_for_queries[b],
            "wk": W_K,
            "wv": W_V,
            "wq": W_Q,
        }
        for b in range(B)
    ]
    out_arrs = fn(*_concat_inputs(in_maps))
    z_all = np.asarray(out_arrs[out_names.index("z")]).astype(np.float32)
    return z_all.reshape(B, S, D)


if __name__ == "__main__":
    rng = np.random.default_rng(0)
    ins = {
        "inputs_for_keys": rng.standard_normal((B, S, D), dtype=np.float32),
        "inputs_for_values": rng.standard_normal((B, S, D), dtype=np.float32),
        "inputs_for_queries": rng.standard_normal((B, S, D), dtype=np.float32),
        "W_K": (rng.standard_normal((D, D)) * 0.05).astype(np.float32),
        "W_V": (rng.standard_normal((D, D)) * 0.05).astype(np.float32),
        "W_Q": (rng.standard_normal((D, D)) * 0.05).astype(np.float32),
    }
    out = kernel(**ins)
    print("out", out.shape, out.dtype)
